# revision 8
# baseline (speedup 1.0000x reference)
"""GCN (4-layer, categorical-encoder, mean-pool) Trainium2 Bass kernel, 8 NeuronCores.

v2: hardware-looped (For_i + dynamic APs) edition.

Sharding: edges partitioned by destination-node range (8 contiguous slices of
6250 nodes). Weights replicated. Per layer: each core computes xw for its node
slice (For_i over 49 blocks), AllGathers xw (bf16) into a DRAM table, then a
For_i over 49 dst blocks gathers that block's dst-sorted edge messages
(dma_gather) and segment-sums them with one-hot matmuls on the PE into a PSUM
accumulator. deg^-1/2 and pool counts are computed on host and shipped as tiny
tables. Mean-pool partials are AllReduced at the end.
"""
import math
import os
import tempfile
import numpy as np
import ml_dtypes

# Persistent XLA compilation cache: the SPMD wrapper around the Bass NEFF is
# re-jitted on every run_bass_kernel_spmd call (fresh closure); caching the
# compiled executable on disk removes that per-call cost.
try:
    import jax
    jax.config.update("jax_compilation_cache_dir",
                      os.path.join(tempfile.gettempdir(), "jaxcache"))
    jax.config.update("jax_persistent_cache_min_compile_time_secs", 0.0)
    jax.config.update("jax_persistent_cache_min_entry_size_bytes", 0)
except Exception:
    pass

import concourse.bass as bass
import concourse.bacc as bacc
import concourse.tile as tile
import concourse.mybir as mybir
from concourse.bass import ds
from concourse.bass_utils import run_bass_kernel_spmd
from concourse.masks import make_identity

BF16 = ml_dtypes.bfloat16

# problem constants (hardcoded per task instructions)
N, E, D, L, G, C, V, O = 50000, 800000, 128, 4, 512, 4, 128, 128
NCOR = 8
P = 128
SLICE = N // NCOR            # 6250 real nodes per core
NBLK = math.ceil(SLICE / P)  # 49 dst blocks per core
SLICE_PAD = NBLK * P         # 6272
AGR = NCOR * SLICE_PAD       # 50176 rows in the allgather table
SPLIT = 32768                # int16 gather-index limit
NGB = G // P                 # 4 graph blocks
CALL_TILES = 8               # max tiles per dma_gather call (1024 idxs)
EMB_CT = 7                   # tiles per embedding gather call (49 = 7*7)
EMB_COLS = C * NBLK * P // 16  # 1568 wrapped idx columns for the embedding


def _chunks(t):
    out = []
    while t > 0:
        c = min(CALL_TILES, t)
        out.append(c)
        t -= c
    return out


def _prep(x, edge_index, batch):
    """Host-side sharding (vectorized): per-core wrapped gather indices,
    one-hot dst labels, dinv / batch / rcnt tables."""
    x = np.asarray(x)
    batch = np.asarray(batch)
    src = np.concatenate([np.asarray(edge_index[0], np.int64),
                          np.arange(N, dtype=np.int64)])
    dst = np.concatenate([np.asarray(edge_index[1], np.int64),
                          np.arange(N, dtype=np.int64)])
    agrow = (src // SLICE) * SLICE_PAD + (src % SLICE)

    blk_g = (dst // SLICE) * NBLK + (dst % SLICE) // P   # global dst block
    dl = (dst % SLICE) % P                               # dst row within block
    hi = agrow >= SPLIT

    # sort by (block, phase, dst-row): dst-sorted slots let the one-hot be
    # reconstructed on device from per-dst count boundaries
    order = np.lexsort((dl, hi, blk_g))
    blk_s, dl_s, hi_s, ag_s = blk_g[order], dl[order], hi[order], agrow[order]

    g2 = blk_s * 2 + hi_s
    counts = np.bincount(g2, minlength=NCOR * NBLK * 2)
    starts = np.concatenate([[0], np.cumsum(counts)[:-1]])
    rank = np.arange(len(g2)) - starts[g2]

    cnt_lo = counts[0::2].reshape(NCOR, NBLK)
    cnt_hi = counts[1::2].reshape(NCOR, NBLK)
    T1 = max(1, -(-int(cnt_lo.max()) // P))
    T2 = max(1, -(-int(cnt_hi.max()) // P))
    TT = T1 + T2

    eidx = np.zeros(NCOR * NBLK * TT * P, np.int16)
    flat = blk_s * (TT * P) + np.where(hi_s, T1 * P + rank, rank)
    eidx[flat] = np.where(hi_s, ag_s - SPLIT, ag_s).astype(np.int16)
    eidx = eidx.reshape(NCOR, NBLK, TT, P)

    # per-(block, phase, dst-row) count boundaries -> [NBLK*4, 128] i16 rows
    # per block: (lo_excl, lo_incl, hi_excl, hi_incl)
    c3 = np.bincount(g2 * P + dl_s, minlength=NCOR * NBLK * 2 * P)
    c3 = c3.reshape(NCOR * NBLK, 2, P)
    incl = np.cumsum(c3, axis=2)
    excl = incl - c3
    cum = np.stack([excl[:, 0], incl[:, 0], excl[:, 1], incl[:, 1]],
                   axis=1).astype(np.int16)           # [NCOR*NBLK, 4, P]
    cum = cum.reshape(NCOR, NBLK * 4, P)

    # gather-call chunking (lo tiles then hi tiles, each <= CALL_TILES)
    call_plan = []   # (phase, t0, ntiles, col_off) within a block
    off = 0
    t0 = 0
    for nt in _chunks(T1):
        call_plan.append((0, t0, nt, off))
        t0 += nt
        off += nt * P // 16
    t0 = T1
    for nt in _chunks(T2):
        call_plan.append((1, t0, nt, off))
        t0 += nt
        off += nt * P // 16

    # wrapped edge index columns: [16, NBLK * TT*8] per core, block-major,
    # chunk-contiguous within a block
    edge_cols = np.empty((NCOR, 16, NBLK, TT * 8), np.int16)
    for (_, t0, nt, off) in call_plan:
        a = eidx[:, :, t0:t0 + nt, :].reshape(NCOR, NBLK, nt * 8, 16)
        edge_cols[:, :, :, off:off + nt * 8] = a.transpose(0, 3, 1, 2)

    # embedding gather indices, wrapped: [16, C*392] per core
    nodes = np.arange(SLICE_PAD)
    emb_cols = np.empty((NCOR, 16, C * NBLK * 8), np.int16)
    for c in range(NCOR):
        xs = np.zeros((SLICE_PAD, C), np.int16)
        xs[:SLICE] = x[c * SLICE:(c + 1) * SLICE]
        ei = (np.arange(C)[:, None] * V + xs.T).astype(np.int16)  # [C, 6272]
        a = ei.reshape(C * NBLK // EMB_CT, EMB_CT * P // 16, 16)
        emb_cols[c] = a.transpose(2, 0, 1).reshape(16, -1)

    # per-node tables
    deg = np.bincount(dst, minlength=N).astype(np.float32)
    dinv_full = np.zeros(NCOR * SLICE_PAD, np.float32)
    batch_full = np.full(NCOR * SLICE_PAD, -1.0, np.float32)
    idx = (np.arange(N) // SLICE) * SLICE_PAD + np.arange(N) % SLICE
    with np.errstate(divide="ignore"):
        dinv_full[idx] = np.where(deg > 0, deg ** -0.5, 0.0)
    batch_full[idx] = batch.astype(np.float32)
    dinv_full = dinv_full.reshape(NCOR, NBLK, P)
    batch_full = batch_full.reshape(NCOR, NBLK, P)

    cnt = np.bincount(batch, minlength=G).astype(np.float32)
    rcnt = 1.0 / np.maximum(cnt, 1.0)   # [512]

    GS = G // NCOR  # 64 graphs output per core
    rsel = rcnt.reshape(NGB, P).T.copy()  # rsel[p, gb] = 1/cnt[gb*128+p]
    per_core = []
    for c in range(NCOR):
        # on-device graph-selection build: gsel_gb[p, j] = (j == bsel[p, gb])
        # * rsel[p, gb], selecting this core's 64 output graphs with the
        # mean-pool 1/cnt folded in
        bsel = np.full((P, NGB), -999.0, np.float32)
        tgt_gb, off = (c * GS) // P, (c * GS) % P
        pr = np.arange(P)
        m = (pr >= off) & (pr < off + GS)
        bsel[m, tgt_gb] = (pr - off)[m]
        per_core.append(dict(
            eidx=np.concatenate([emb_cols[c], edge_cols[c].reshape(16, -1)],
                                axis=1),
            cum=np.ascontiguousarray(cum[c]),                   # [NBLK*4, 128]
            dinv=np.ascontiguousarray(dinv_full[c].T),          # [128, NBLK]
            batchv=np.ascontiguousarray(batch_full[c].T).astype(np.int16),
            bsel=bsel, rsel=rsel,
        ))
    static = dict(T1=T1, T2=T2, call_plan=tuple(call_plan),
                  eidx_cols=per_core[0]["eidx"].shape[1])
    return per_core, static


def _build(static, weights, repeat=1):
    T1, T2 = static["T1"], static["T2"]
    TT = T1 + T2
    NT = NBLK * TT
    call_plan = static["call_plan"]
    eidx_cols = static["eidx_cols"]

    nc = bacc.Bacc("TRN2", target_bir_lowering=False, debug=False,
                   num_devices=NCOR)
    f32, bf16, i16 = mybir.dt.float32, mybir.dt.bfloat16, mybir.dt.int16
    i8 = mybir.dt.int8
    GS = G // NCOR

    eidx_in = nc.dram_tensor("eidx", [16, eidx_cols], i16, kind="ExternalInput")
    cum_in = nc.dram_tensor("cum", [NBLK * 4, P], i16, kind="ExternalInput")
    dinv_in = nc.dram_tensor("dinv", [P, NBLK], f32, kind="ExternalInput")
    batchv_in = nc.dram_tensor("batchv", [P, NBLK], i16, kind="ExternalInput")
    bsel_in = nc.dram_tensor("bsel", [P, NGB], f32, kind="ExternalInput")
    rsel_in = nc.dram_tensor("rsel", [P, NGB], f32, kind="ExternalInput")
    # weights are identical on every core: bake them into the NEFF as inline
    # consts instead of shipping 8 copies over the tunnel per call
    wmat_in = nc.inline_tensor(weights["wmat"], name="wmat")
    wr_in = nc.inline_tensor(weights["wr"], name="wr")
    bias_in = nc.inline_tensor(weights["biasrow"], name="biasrow")
    embt_in = nc.inline_tensor(weights["embt"], name="embt")
    out_t = nc.dram_tensor("out", [GS, O], f32, kind="ExternalOutput")
    # Shared-scratchpad collective outputs (fast path for HBM-HBM collectives)
    ag_out_h = nc.dram_tensor("ag_out_sh", [AGR, D], bf16, kind="Internal",
                              addr_space="Shared")
    ar_out_h = nc.dram_tensor("ar_out_sh", [P, NGB * D], f32, kind="Internal",
                              addr_space="Shared")

    with tile.TileContext(nc) as tc:
        with tc.tile_pool(name="const", bufs=1) as cp, \
             tc.tile_pool(name="dram", bufs=1, space="DRAM") as dram, \
             tc.tile_pool(name="state", bufs=1) as sp:
            # ---- constants into SBUF ----
            eidx_s = cp.tile([P, eidx_cols], i16, tag="eidx")
            nc.sync.dma_start(eidx_s[0:16, :], eidx_in[:])
            # replicate the 16-partition wrap to all 128 partitions (3 doublings)
            nc.sync.dma_start(eidx_s[16:32, :], eidx_s[0:16, :])
            nc.sync.dma_start(eidx_s[32:64, :], eidx_s[0:32, :])
            nc.sync.dma_start(eidx_s[64:128, :], eidx_s[0:64, :])
            dinv_s = cp.tile([P, NBLK], f32, tag="dinv")
            nc.sync.dma_start(dinv_s[:], dinv_in[:])
            batchv_i16 = cp.tile([P, NBLK], i16, tag="batchvi")
            nc.sync.dma_start(batchv_i16[:], batchv_in[:])
            batchv_s = cp.tile([P, NBLK], f32, tag="batchv")
            nc.vector.tensor_copy(out=batchv_s[:], in_=batchv_i16[:])
            iota_s = cp.tile([P, P], bf16, tag="iota")
            nc.gpsimd.iota(iota_s[:], pattern=[[1, P]], base=0,
                           channel_multiplier=0,
                           allow_small_or_imprecise_dtypes=True)
            ident_s = cp.tile([P, P], bf16, tag="ident")
            make_identity(nc, ident_s[:])
            # gcol[p, t] = t*128 + p : slot index within a phase segment
            TG = max(T1, T2)
            gcol_s = cp.tile([P, TG], f32, tag="gcol")
            nc.gpsimd.iota(gcol_s[:], pattern=[[P, TG]], base=0,
                           channel_multiplier=1,
                           allow_small_or_imprecise_dtypes=True)
            # row-selector lhsT tiles: sel4[:, r*P:(r+1)*P] broadcasts
            # partition r of a 4-row tile to all 128 output partitions.
            # Block-diagonal band sel4[p, col] = (col//P == p), built with
            # affine_select (v = col - P*p; keep where 0 <= v <= P-1).
            sel4 = cp.tile([4, 4 * P], f32, tag="sel4")
            nc.gpsimd.memset(sel4[:], 1.0)
            nc.gpsimd.affine_select(
                out=sel4[:], in_=sel4[:], pattern=[[1, 4 * P]],
                compare_op=mybir.AluOpType.is_ge, fill=0.0, base=0,
                channel_multiplier=-P)
            nc.gpsimd.affine_select(
                out=sel4[:], in_=sel4[:], pattern=[[-1, 4 * P]],
                compare_op=mybir.AluOpType.is_ge, fill=0.0, base=P - 1,
                channel_multiplier=P)
            bsel_s = cp.tile([P, NGB], f32, tag="bsel")
            nc.sync.dma_start(bsel_s[:], bsel_in[:])
            rsel_s = cp.tile([P, NGB], f32, tag="rsel")
            nc.sync.dma_start(rsel_s[:], rsel_in[:])
            gsel_s = cp.tile([P, NGB * GS], bf16, tag="gsel")
            for gb in range(NGB):
                ohg = cp.tile([P, GS], bf16, tag=f"ohg{gb}")
                nc.vector.tensor_scalar(
                    out=ohg[:], in0=iota_s[:, 0:GS],
                    scalar1=bsel_s[:, gb:gb + 1], scalar2=None,
                    op0=mybir.AluOpType.is_equal)
                nc.vector.tensor_scalar(
                    out=gsel_s[:, gb * GS:(gb + 1) * GS], in0=ohg[:],
                    scalar1=rsel_s[:, gb:gb + 1], scalar2=None,
                    op0=mybir.AluOpType.mult)
            w_bf = cp.tile([P, L * D], bf16, tag="wbf")
            nc.sync.dma_start(w_bf[:], wmat_in[:])
            wr_s = cp.tile([P, O], bf16, tag="wr")
            nc.sync.dma_start(wr_s[:], wr_in[:])
            # broadcast bias row to all 128 partitions via a rank-1 matmul
            brow = cp.tile([P, L * D + O], f32, tag="brow")
            nc.vector.memset(brow[:], 0.0)
            nc.sync.dma_start(brow[0:1, :], bias_in[:])
            row1 = cp.tile([P, P], bf16, tag="row1")
            nc.vector.memset(row1[:], 0.0)
            nc.vector.memset(row1[0:1, :], 1.0)
            brow_bf = cp.tile([P, L * D + O], bf16, tag="browbf")
            nc.vector.tensor_copy(out=brow_bf[:], in_=brow[:])
            bb_s = cp.tile([P, L * D], f32, tag="bb")
            brb_s = cp.tile([P, O], f32, tag="brb")
            with tc.tile_pool(name="bcast", bufs=1, space="PSUM") as bp:
                for j in range(L + 1):
                    pb = bp.tile([P, D], f32, tag=f"pb{j}", space="PSUM")
                    nc.tensor.matmul(out=pb[:], lhsT=row1[:],
                                     rhs=brow_bf[:, j * D:(j + 1) * D],
                                     start=True, stop=True)
                    if j < L:
                        nc.vector.tensor_copy(out=bb_s[:, j * D:(j + 1) * D],
                                              in_=pb[:])
                    else:
                        nc.vector.tensor_copy(out=brb_s[:], in_=pb[:])
            # shifted batch values for the 4 graph blocks (one-hot scalars)
            bsh_s = cp.tile([P, NGB * NBLK], f32, tag="bsh")
            for gb in range(NGB):
                nc.vector.tensor_scalar(
                    out=bsh_s[:, gb * NBLK:(gb + 1) * NBLK], in0=batchv_s[:],
                    scalar1=float(gb * P), scalar2=None,
                    op0=mybir.AluOpType.subtract)

            # ---- DRAM comm buffers ----
            ag_in = dram.tile([SLICE_PAD, D], bf16, tag="ag_in")
            ar_in = dram.tile([P, NGB * D], f32, tag="ar_in")

            # ---- persistent state ----
            h_s = sp.tile([P, NBLK * D], f32, tag="h")
            xs_bf = sp.tile([P, NBLK * D], bf16, tag="xsbf")
            xw_bf = sp.tile([P, NBLK * D], bf16, tag="xwbf")

            for rep in range(repeat):
                rp = f"r{rep}"
                # ============ embedding ============
                with tc.tile_pool(name="embp", bufs=2) as ep:
                    for col in range(C):
                        reg = ep.tile([P, NBLK, D], bf16, tag="embreg",
                                      name=f"emb{rp}_{col}")
                        for k in range(NBLK // EMB_CT):
                            cbase = (col * (NBLK // EMB_CT) + k) * EMB_CT * P // 16
                            nc.gpsimd.dma_gather(
                                out_ap=reg[:, k * EMB_CT:(k + 1) * EMB_CT, :],
                                in_ap=embt_in[:],
                                idxs_ap=eidx_s[:, cbase:cbase + EMB_CT * P // 16],
                                num_idxs=EMB_CT * P, num_idxs_reg=EMB_CT * P,
                                elem_size=D)
                        r2 = reg[:].rearrange("p t d -> p (t d)")
                        if col == 0:
                            nc.vector.tensor_copy(out=h_s[:], in_=r2)
                        else:
                            nc.vector.tensor_tensor(out=h_s[:], in0=h_s[:],
                                                    in1=r2,
                                                    op=mybir.AluOpType.add)
                    with tc.For_i(0, NBLK, 1) as nt:
                        nc.vector.tensor_scalar(
                            out=xs_bf[:, ds(nt * D, D)],
                            in0=h_s[:, ds(nt * D, D)],
                            scalar1=dinv_s[:, ds(nt, 1)], scalar2=None,
                            op0=mybir.AluOpType.mult)

                # ============ layers ============
                for l in range(L):
                    # ---- xs -> xw (For_i over blocks) -> allgather ----
                    with tc.tile_pool(name="xwp", bufs=1, space="PSUM") as xwp, \
                         tc.tile_pool(name="xst", bufs=1) as xst:
                        with tc.For_i(0, NBLK, 1) as nt:
                            stage = xst.tile([P, P], bf16, tag="xstage")
                            nc.vector.tensor_copy(out=stage[:],
                                                  in_=xs_bf[:, ds(nt * D, D)])
                            psT = xwp.tile([P, P], bf16, tag="psT", space="PSUM")
                            nc.tensor.transpose(out=psT[:], in_=stage[:],
                                                identity=ident_s[:])
                            xsT = xst.tile([P, P], bf16, tag="xsT")
                            nc.vector.tensor_copy(out=xsT[:], in_=psT[:])
                            psW = xwp.tile([P, P], f32, tag="psW", space="PSUM")
                            nc.tensor.matmul(out=psW[:], lhsT=xsT[:],
                                             rhs=w_bf[:, l * D:(l + 1) * D],
                                             start=True, stop=True)
                            nc.vector.tensor_copy(out=xw_bf[:, ds(nt * D, D)],
                                                  in_=psW[:])
                        nc.sync.dma_start(
                            ag_in[:].rearrange("(t p) d -> p t d", p=P),
                            xw_bf[:].rearrange("p (t d) -> p t d", d=D))
                        nc.gpsimd.collective_compute(
                            "AllGather", mybir.AluOpType.bypass,
                            replica_groups=[list(range(NCOR))],
                            ins=[ag_in.opt()], outs=[ag_out_h[:].opt()])

                    # ---- gather + aggregate + epilogue (For_i over blocks) ----
                    with tc.tile_pool(name="msgp", bufs=1) as msgp, \
                         tc.tile_pool(name="aggp", bufs=1, space="PSUM") as aggp, \
                         tc.tile_pool(name="ohp", bufs=2) as ohp, \
                         tc.tile_pool(name="epi", bufs=1) as epi:
                        with tc.For_i(0, NBLK, 1) as b:
                            msg = msgp.tile([P, TT, D], bf16, tag="msg")
                            for (phase, t0, ntc, coff) in call_plan:
                                src_ap = (ag_out_h[:SPLIT, :] if phase == 0
                                          else ag_out_h[SPLIT:, :])
                                nc.gpsimd.dma_gather(
                                    out_ap=msg[:, t0:t0 + ntc, :], in_ap=src_ap,
                                    idxs_ap=eidx_s[:, ds(EMB_COLS + b * (TT * 8)
                                                         + coff, ntc * 8)],
                                    num_idxs=ntc * P, num_idxs_reg=ntc * P,
                                    elem_size=D)
                            # broadcast this block's 4 boundary rows
                            # (lo_excl, lo_incl, hi_excl, hi_incl) to all
                            # partitions: thr[:, r*P+j] = cum[b, r, j]
                            cum_i = ohp.tile([4, P], i16, tag="cumi")
                            nc.sync.dma_start(cum_i[:], cum_in[ds(b * 4, 4), :])
                            cum_f = ohp.tile([4, P], f32, tag="cumf")
                            nc.vector.tensor_copy(out=cum_f[:], in_=cum_i[:])
                            thr_ps = aggp.tile([P, 4 * P], f32, tag="thr",
                                               space="PSUM")
                            for r in range(4):
                                nc.tensor.matmul(
                                    out=thr_ps[:, r * P:(r + 1) * P],
                                    lhsT=sel4[:, r * P:(r + 1) * P],
                                    rhs=cum_f[:], start=True, stop=True)
                            thr = ohp.tile([P, 4 * P], f32, tag="thrs")
                            nc.vector.tensor_copy(out=thr[:], in_=thr_ps[:])
                            ps = aggp.tile([P, P], f32, tag="agg", space="PSUM")
                            for t in range(TT):
                                if t < T1:
                                    exc, inc = thr[:, 0:P], thr[:, P:2 * P]
                                    gc = gcol_s[:, t:t + 1]
                                else:
                                    exc, inc = thr[:, 2 * P:3 * P], thr[:, 3 * P:4 * P]
                                    gc = gcol_s[:, t - T1:t - T1 + 1]
                                # one-hot: excl[j] <= slot_p < incl[j]
                                a1 = ohp.tile([P, P], bf16, tag="oha",
                                              name=f"oha{t}")
                                nc.vector.tensor_scalar(
                                    out=a1[:], in0=exc, scalar1=gc,
                                    scalar2=None, op0=mybir.AluOpType.is_le)
                                oh = ohp.tile([P, P], bf16, tag="oh",
                                              name=f"oh{t}")
                                nc.vector.scalar_tensor_tensor(
                                    out=oh[:], in0=inc, scalar=gc,
                                    in1=a1[:], op0=mybir.AluOpType.is_gt,
                                    op1=mybir.AluOpType.mult)
                                nc.tensor.matmul(out=ps[:], lhsT=oh[:],
                                                 rhs=msg[:, t, :],
                                                 start=(t == 0),
                                                 stop=(t == TT - 1))
                            t2t = epi.tile([P, P], f32, tag="t2")
                            nc.vector.scalar_tensor_tensor(
                                out=t2t[:], in0=ps[:],
                                scalar=dinv_s[:, ds(b, 1)],
                                in1=bb_s[:, l * D:(l + 1) * D],
                                op0=mybir.AluOpType.mult,
                                op1=mybir.AluOpType.add)
                            if l < L - 1:
                                nc.vector.tensor_scalar(
                                    out=xs_bf[:, ds(b * D, D)], in0=t2t[:],
                                    scalar1=0.0, scalar2=dinv_s[:, ds(b, 1)],
                                    op0=mybir.AluOpType.max,
                                    op1=mybir.AluOpType.mult)
                            else:
                                nc.vector.tensor_scalar(
                                    out=xs_bf[:, ds(b * D, D)], in0=t2t[:],
                                    scalar1=0.0, scalar2=None,
                                    op0=mybir.AluOpType.max)

                # ============ mean-pool ============
                with tc.tile_pool(name="finp", bufs=1) as fp:
                    # acc[graph-in-block, gb*D + feat] = pooled sums
                    acc = fp.tile([P, NGB * D], f32, tag="acc")
                    nc.vector.memset(acc[:], 0.0)
                    with tc.tile_pool(name="poolp", bufs=1, space="PSUM") as pp, \
                         tc.tile_pool(name="pohp", bufs=1) as pohp:
                        with tc.For_i(0, NBLK, 1) as nt:
                            for gb in range(NGB):
                                oh = pohp.tile([P, P], bf16, tag=f"poh{gb}")
                                nc.vector.tensor_scalar(
                                    out=oh[:], in0=iota_s[:],
                                    scalar1=bsh_s[:, ds(gb * NBLK + nt, 1)],
                                    scalar2=None, op0=mybir.AluOpType.is_equal)
                                psg = pp.tile([P, P], f32, tag=f"psg{gb}",
                                              space="PSUM")
                                nc.tensor.matmul(
                                    out=psg[:], lhsT=oh[:],
                                    rhs=xs_bf[:, ds(nt * D, D)],
                                    start=True, stop=True)
                                nc.vector.tensor_tensor(
                                    out=acc[:, gb * D:(gb + 1) * D],
                                    in0=acc[:, gb * D:(gb + 1) * D],
                                    in1=psg[:], op=mybir.AluOpType.add)
                    nc.sync.dma_start(ar_in[:], acc[:])
                    nc.gpsimd.collective_compute(
                        "AllReduce", mybir.AluOpType.add,
                        replica_groups=[list(range(NCOR))],
                        ins=[ar_in.opt()], outs=[ar_out_h[:].opt()])
                    arr = fp.tile([P, NGB * D], f32, tag="arr")
                    nc.sync.dma_start(arr[:], ar_out_h[:])
                    arr_bf = fp.tile([P, NGB * D], bf16, tag="arrbf")
                    nc.vector.tensor_copy(out=arr_bf[:], in_=arr[:])
                    with tc.tile_pool(name="outp", bufs=1, space="PSUM") as op_:
                        # selT[feat, j] = mean-pooled g[c*GS+j, feat]
                        # (gsel carries the 1/cnt mean factor)
                        selT = op_.tile([P, GS], f32, tag="selT",
                                        name=f"selT{rp}", space="PSUM")
                        for gb in range(NGB):
                            nc.tensor.matmul(
                                out=selT[:],
                                lhsT=arr_bf[:, gb * D:(gb + 1) * D],
                                rhs=gsel_s[:, gb * GS:(gb + 1) * GS],
                                start=(gb == 0), stop=(gb == NGB - 1))
                        selT_bf = fp.tile([P, GS], bf16, tag="selTbf",
                                          name=f"selTbf{rp}")
                        nc.vector.tensor_copy(out=selT_bf[:], in_=selT[:])
                        pso = op_.tile([GS, O], f32, tag="pso",
                                       name=f"pso{rp}", space="PSUM")
                        nc.tensor.matmul(out=pso[:], lhsT=selT_bf[:],
                                         rhs=wr_s[:], start=True, stop=True)
                        o1 = fp.tile([GS, O], f32, tag="o1", name=f"o1{rp}")
                        nc.vector.tensor_tensor(
                            out=o1[:], in0=pso[:], in1=brb_s[0:GS, :],
                            op=mybir.AluOpType.add)
                        nc.sync.dma_start(out_t[:], o1[:])
    nc.compile()
    # bass2jax re-serializes the BIR on every lowering (once per
    # run_bass_kernel_spmd call); the module is frozen after compile(), so
    # memoize the serialization.
    try:
        frozen_json = nc.to_json_bytes()
        nc.to_json_bytes = lambda: frozen_json
    except Exception:
        pass
    return nc


_CACHE = {}


def _weights(emb, W, b, Wr, br):
    return dict(
        wmat=np.concatenate([np.asarray(W, np.float32)[l] for l in range(L)],
                            axis=1).astype(BF16),
        wr=np.asarray(Wr, np.float32).astype(BF16),
        biasrow=np.concatenate([np.asarray(b, np.float32).ravel(),
                                np.asarray(br, np.float32)]).reshape(1, -1),
        embt=np.asarray(emb, np.float32).reshape(C * V, D).astype(BF16),
    )


def _get_nc(static, weights, repeat=1):
    import hashlib
    h = hashlib.sha256()
    for k in sorted(weights):
        h.update(np.ascontiguousarray(weights[k]).tobytes())
    key = (static["T1"], static["T2"], static["eidx_cols"], repeat,
           h.hexdigest())
    if key not in _CACHE:
        _CACHE[key] = _build(static, weights, repeat)
    return _CACHE[key]


def _make_in_maps(per_core):
    in_maps = []
    for c in range(NCOR):
        in_maps.append(dict(
            eidx=per_core[c]["eidx"], cum=per_core[c]["cum"],
            dinv=per_core[c]["dinv"], batchv=per_core[c]["batchv"],
            bsel=per_core[c]["bsel"], rsel=per_core[c]["rsel"]))
    return in_maps


def kernel(x, edge_index, batch, emb, W, b, Wr, br, _repeat=1):
    per_core, static = _prep(np.asarray(x), np.asarray(edge_index),
                             np.asarray(batch))
    nc = _get_nc(static, _weights(emb, W, b, Wr, br), _repeat)
    in_maps = _make_in_maps(per_core)
    res = run_bass_kernel_spmd(nc, in_maps, core_ids=list(range(NCOR)))
    return np.concatenate([res.results[c]["out"] for c in range(NCOR)],
                          axis=0).astype(np.float32)
